# revision 1
# baseline (speedup 1.0000x reference)
"""Trainium2 Bass kernel for nn_MiddleOut (gnn_message_passing).

Math (reference):
    out[b,r] = mean_p[ m[b,p] * (my@Wm.T + bias + peer[b,p]@Wp.T + m[b,p]*wm)[r] ]
Collapses to (P = #peers):
    s1[b] = sum_p m[b,p];  s2[b] = sum_p m[b,p]^2
    z[b,l] = sum_p m[b,p] * peer[b,p,l]
    out = (1/P) * [ (s1*my) | z | s2 | s1 ] @ [ Wm.T ; Wp.T ; wm ; bias ]

Sharding: pure data parallel over batch across 8 cores.

On-device strategy per core (Bc=2048 rows, 16 tiles of 128):
  - peer tile host-permuted to [(b4,p)=128 partitions, g=32 groups, l=256]
    (batch b_local = g*4 + b4), cast to bf16 on host (memory-bound problem:
    halves the dominant stream; out rel err ~4e-4), each tile one contiguous
    2MB block so DMA moves 16KB runs per partition.
  - The weighted peer-reduction z runs on the TensorEngine: per group g the
    [128,128] stationary S holds m[g*4+b4, p] at column 4g+b4, rows (b4,p)
    (a zeroed ping-pong tile whose stride-132 diagonal band is rewritten by
    4 DVE copies per tile), so 32 chained matmuls PSUM-accumulate
    psum_z[b_local, l] = sum_p m * peer in natural batch order.
  - s1/s2 from DVE reduce ops, u = s1*my via tensor_scalar.
  - X = [u | z] is PE-transposed in 128-col chunks (fp32-exact), evacuated by
    ACT copies that round to float32r, and fed as stationary into a K=514
    float32r matmul (1 cyc/col vs fp32's 4) against the host-prepacked
    [Wm.T; Wp.T; wm; bias] moving operand, accumulating straight to out.
  - DMA issue is split across the two HWDGE engines (sync: x, scalar:
    meta/out); mt/mb/my are packed into one meta tensor per tile.
"""

import ml_dtypes
import numpy as np

import concourse.bass as bass
import concourse.mybir as mybir
import concourse.tile as tile
from concourse import bacc
from concourse.bass_utils import run_bass_kernel_spmd

F32 = mybir.dt.float32
F32R = mybir.dt.float32r

B, P, L, R = 16384, 32, 256, 256
N_CORES = 8
BC = B // N_CORES          # 2048 batches per core
TILE_B = 128               # batches per SBUF tile
NT = BC // TILE_B          # 16 tiles
G = TILE_B // 4            # 32 groups of 4 batches
NK = 4                     # 128-wide feature chunks of [u|z]


PRECISION = "bf16"   # "f32r": PE-heavy reduced-precision matmuls; "f32": exact


def is_pe_tile(t):
    """Tiles whose peer-reduction runs on the TensorEngine; the rest run a
    DVE multiply-accumulate chain so both engines stay under the DMA floor."""
    if PRECISION in ("f32r", "bf16"):
        return True
    return t % 3 == 0

_cache = {}


def build_bass(nt=NT, num_devices=N_CORES):
    bc = nt * TILE_B
    nc = bacc.Bacc(
        "TRN2", target_bir_lowering=False, debug=False, num_devices=num_devices
    )

    FR = F32R if PRECISION in ("f32r", "bf16") else F32
    BF = mybir.dt.bfloat16
    XD = BF if PRECISION == "bf16" else FR
    x_d = nc.dram_tensor("x", [nt, TILE_B, G, L], XD, kind="ExternalInput")
    # meta packs [mt | mb | my] per tile: one DMA instead of three
    meta_d = nc.dram_tensor(
        "meta", [nt, TILE_B, G + P + L], F32, kind="ExternalInput"
    )
    w_d = nc.dram_tensor("wext", [5, TILE_B, R], FR, kind="ExternalInput")
    id_d = nc.dram_tensor("ident", [TILE_B, TILE_B], F32, kind="ExternalInput")
    out_d = nc.dram_tensor("out", [bc, R], F32, kind="ExternalOutput")

    with TileCtx(nc) as (tc, ctx):
        singles = ctx.enter_context(tc.tile_pool(name="singles", bufs=1))
        xp = ctx.enter_context(tc.tile_pool(name="xp", bufs=6))
        small = ctx.enter_context(tc.tile_pool(name="small", bufs=6))
        xtp = ctx.enter_context(tc.tile_pool(name="xtp", bufs=4))
        psz = ctx.enter_context(tc.tile_pool(name="psz", bufs=3, space="PSUM"))
        pst = ctx.enter_context(tc.tile_pool(name="pst", bufs=2, space="PSUM"))
        pso = ctx.enter_context(tc.tile_pool(name="pso", bufs=3, space="PSUM"))

        w_sb = singles.tile([TILE_B, 5, R], FR)
        nc.sync.dma_start(out=w_sb, in_=w_d.rearrange("k p r -> p k r"))
        ident = singles.tile([TILE_B, TILE_B], F32)
        nc.sync.dma_start(out=ident, in_=id_d[:, :])

        # Ping-pong block-diagonal stationaries for the weighted peer-reduce.
        # s[:, g, :] is [128, 128]: column 4g+b4 holds m[g*4+b4, p] at rows
        # (b4, p); the zeros are written once, the diagonal band is rewritten
        # every tile. f32r matmuls need the full M=128 stationary.
        s_tiles = []
        for i in range(3):
            s_i = singles.tile([TILE_B, G, TILE_B], XD, tag=f"s{i}")
            if PRECISION == "bf16":
                nc.vector.memset(s_i, 0.0)
            else:
                nc.vector.memset(s_i.bitcast(F32), 0.0)
            s_tiles.append(s_i)

        for t in range(nt):
            # ---- loads ----
            if is_pe_tile(t):
                x_t = xp.tile([TILE_B, G, L], XD, tag="x_t")
                nc.sync.dma_start(out=x_t[:, 0:G // 2, :], in_=x_d[t, :, 0:G // 2, :])
                nc.sync.dma_start(out=x_t[:, G // 2:, :], in_=x_d[t, :, G // 2:, :])
            elif PRECISION == "bf16":
                x_t = xp.tile([TILE_B, G, L], BF, tag="x_t")
                nc.sync.dma_start(out=x_t, in_=x_d[t])
            else:
                x_t = xp.tile([TILE_B, G, L], F32, tag="x_t")
                nc.sync.dma_start(out=x_t, in_=x_d[t].bitcast(F32))
            meta = small.tile([TILE_B, G + P + L], F32, tag="meta")
            nc.scalar.dma_start(out=meta, in_=meta_d[t])
            m_t = meta[:, 0:G]
            m_b = meta[:, G:G + P]
            my_t = meta[:, G + P:]

            psum_z = None
            if is_pe_tile(t):
                # ---- fill the diagonal band of S with this tile's metrics ----
                s_all = s_tiles[t % 3]
                for b4 in range(4):
                    view = s_all[b4 * P:(b4 + 1) * P, :, :]
                    out_ap = bass.AP(
                        tensor=view.tensor, offset=view.offset + b4,
                        ap=[view.ap[0], [132, G]],
                    )
                    nc.vector.tensor_copy(
                        out=out_ap, in_=m_t[b4 * P:(b4 + 1) * P, :],
                    )

                # ---- z via PE: psum_z[b_local, l] = sum_p m * peer ----
                # one 32-matmul f32r accumulation chain, M=128
                psum_z = psz.tile([TILE_B, L], F32, tag="psum_z")
                for g in range(G):
                    nc.tensor.matmul(
                        out=psum_z,
                        lhsT=s_all[:, g, :],
                        rhs=x_t[:, g, :],
                        start=(g == 0),
                        stop=(g == G - 1),
                    )

            # ---- s1, s2, u ----
            s12 = small.tile([TILE_B, 2], F32, tag="s12")  # [s2 | s1]
            m2 = small.tile([TILE_B, P], F32, tag="m2")
            nc.vector.tensor_mul(m2, m_b, m_b)
            nc.vector.tensor_reduce(
                out=s12[:, 0:1], in_=m2, axis=mybir.AxisListType.X,
                op=mybir.AluOpType.add,
            )
            nc.vector.tensor_reduce(
                out=s12[:, 1:2], in_=m_b, axis=mybir.AxisListType.X,
                op=mybir.AluOpType.add,
            )

            x_sb = small.tile([TILE_B, 2 * L], F32, tag="x_sb")  # [u | z]
            nc.vector.tensor_scalar_mul(
                out=x_sb[:, 0:L], in0=my_t, scalar1=s12[:, 1:2]
            )
            if is_pe_tile(t):
                nc.scalar.copy(out=x_sb[:, L:2 * L], in_=psum_z)
            else:
                # ---- z via DVE: two interleaved MAC chains (plain [b,p,l]) ----
                acc0 = small.tile([TILE_B, L], F32, tag="acc0")
                acc1 = small.tile([TILE_B, L], F32, tag="acc1")
                nc.vector.tensor_scalar_mul(
                    out=acc0, in0=x_t[:, 0, :], scalar1=m_b[:, 0:1]
                )
                nc.vector.tensor_scalar_mul(
                    out=acc1, in0=x_t[:, 1, :], scalar1=m_b[:, 1:2]
                )
                for p in range(2, P):
                    acc = acc0 if p % 2 == 0 else acc1
                    nc.vector.scalar_tensor_tensor(
                        out=acc, in0=x_t[:, p, :], scalar=m_b[:, p:p + 1],
                        in1=acc, op0=mybir.AluOpType.mult,
                        op1=mybir.AluOpType.add,
                    )
                nc.vector.tensor_add(x_sb[:, L:2 * L], acc0, acc1)

            # ---- transpose X chunks, matmul against packed weights ----
            xts = []
            for k in range(NK):
                pt = pst.tile([TILE_B, TILE_B], F32, tag="pt")
                nc.tensor.transpose(
                    out=pt, in_=x_sb[:, k * TILE_B:(k + 1) * TILE_B],
                    identity=ident,
                )
                xt = xtp.tile([TILE_B, TILE_B], FR, tag=f"xt{k}")
                nc.scalar.copy(out=xt, in_=pt)
                xts.append(xt)
            pt4 = pst.tile([TILE_B, TILE_B], F32, tag="pt")
            nc.tensor.transpose(out=pt4[0:2, :], in_=s12, identity=ident)
            xt4 = xtp.tile([TILE_B, TILE_B], FR, tag="xt4")
            nc.scalar.copy(out=xt4[0:2, :], in_=pt4[0:2, :])

            psum_o = pso.tile([TILE_B, R], F32, tag="psum_o")
            for k in range(NK):
                nc.tensor.matmul(
                    out=psum_o, lhsT=xts[k], rhs=w_sb[:, k, :],
                    start=(k == 0), stop=False,
                )
            nc.tensor.matmul(
                out=psum_o, lhsT=xt4[0:2, :], rhs=w_sb[0:2, 4, :],
                start=False, stop=True,
            )

            out_sb = small.tile([TILE_B, R], F32, tag="out_sb")
            nc.scalar.activation(
                out=out_sb, in_=psum_o,
                func=mybir.ActivationFunctionType.Copy, scale=1.0 / P,
            )
            nc.scalar.dma_start(
                out=out_d[t * TILE_B:(t + 1) * TILE_B, :], in_=out_sb
            )

    nc.compile()
    return nc


class TileCtx:
    """with TileCtx(nc) as (tc, ctx): — TileContext plus an ExitStack."""

    def __init__(self, nc):
        from contextlib import ExitStack
        self.tc = tile.TileContext(nc)
        self.ctx = ExitStack()

    def __enter__(self):
        return self.tc.__enter__(), self.ctx.__enter__()

    def __exit__(self, *a):
        self.ctx.__exit__(*a)
        return self.tc.__exit__(*a)


def prep_inputs(my_latent, peer_latents, peer_metrics, W, b):
    """Host-side shard + layout prep (no arithmetic beyond weight packing)."""
    wext = np.zeros((5, TILE_B, R), dtype=np.float32)
    wt = np.ascontiguousarray(W.T)                       # [513, 256]
    wext.reshape(5 * TILE_B, R)[0:2 * L] = wt[0:2 * L]
    wext[4, 0] = W[:, 2 * L]                             # wm
    wext[4, 1] = b                                       # bias
    ident = np.eye(TILE_B, dtype=np.float32)

    in_maps = []
    for c in range(N_CORES):
        sl = slice(c * BC, (c + 1) * BC)
        # Each tile is one contiguous 4MB block (32KB per partition row).
        # PE tiles: [(b4,p)=128 partitions, g, l]; DVE tiles: plain [b, p, l].
        xdt = ml_dtypes.bfloat16 if PRECISION == "bf16" else np.float32
        plain = peer_latents[sl].reshape(NT, TILE_B, P, L)
        xc = np.empty((NT, TILE_B, G, L), dtype=xdt)
        for t in range(NT):
            if is_pe_tile(t):
                xc[t] = plain[t].reshape(G, 4, P, L).transpose(
                    1, 2, 0, 3).reshape(TILE_B, G, L)
            else:
                xc[t] = plain[t]
        mc = peer_metrics[sl]
        meta = np.empty((NT, TILE_B, G + P + L), dtype=np.float32)
        meta[:, :, 0:G] = mc.reshape(NT, G, 4, P).transpose(
            0, 2, 3, 1).reshape(NT, TILE_B, G)
        meta[:, :, G:G + P] = mc.reshape(NT, TILE_B, P)
        meta[:, :, G + P:] = my_latent[sl].reshape(NT, TILE_B, L)
        in_maps.append({
            "x": xc,
            "meta": meta,
            "wext": wext,
            "ident": ident,
        })
    return in_maps


def run(my_latent, peer_latents, peer_metrics, W, b, trace=False, **kw):
    if "nc" not in _cache:
        _cache["nc"] = build_bass()
    nc = _cache["nc"]
    in_maps = prep_inputs(
        np.asarray(my_latent, dtype=np.float32),
        np.asarray(peer_latents, dtype=np.float32),
        np.asarray(peer_metrics, dtype=np.float32),
        np.asarray(W, dtype=np.float32),
        np.asarray(b, dtype=np.float32),
    )
    res = run_bass_kernel_spmd(
        nc, in_maps, core_ids=list(range(N_CORES)), trace=trace, **kw
    )
    out = np.concatenate([r["out"] for r in res.results], axis=0)
    return out, res


def kernel(my_latent, peer_latents, peer_metrics, W, b):
    out, _ = run(my_latent, peer_latents, peer_metrics, W, b)
    return out



# revision 8
# speedup vs baseline: 1.3568x; 1.3568x over previous
"""Trainium2 Bass kernel for nn_MiddleOut (gnn_message_passing).

Math (reference):
    out[b,r] = mean_p[ m[b,p] * (my@Wm.T + bias + peer[b,p]@Wp.T + m[b,p]*wm)[r] ]
Collapses to (P = #peers):
    s1[b] = sum_p m[b,p];  s2[b] = sum_p m[b,p]^2
    z[b,l] = sum_p m[b,p] * peer[b,p,l]
    out = s1*(my@Wm.T)/P + z@(Wp.T/P) + s2*(wm/P) + s1*(bias/P)

Sharding: pure data parallel over batch across 8 cores (2048 rows each,
16 tiles of 128).

v2 design (memory-regime: minimize HBM bytes, keep PE under the DMA floor):
  - peer tile host-cast to fp8 e3m4 (1MB/tile, 1/4 of f32); metrics stay
    bf16 so only the x stream is quantized (~3% elem err -> out ~4e-3).
  - z via PE on most tiles: block-diag bf16 metric band stationary
    (rewritten per tile, ping-pong x3) x fp8 moving, 32-matmul PSUM chain.
  - z via DVE stt MAC chains on Z_DVE tiles (plain [b,p,l] layout) to keep
    PE below the DMA roofline; their epilogues are deferred one tile so PE
    never waits on the DVE chain.
  - stage-A prep (band writes on ACT, s1/s2 on DVE) runs one tile ahead so
    a DVE z-chain never blocks the next tile's prep.
  - my-part needs no transposes: host packs myT (bf16) into meta; two
    matmuls with meta slices as stationaries accumulate my@Wm.T/P.
  - z transposed via PE (2x128-col chunks, fp32-exact) + ACT f32r casts;
    final z/s12 matmuls in f32r against host-prepacked Wp.T/P, wm/P, bias/P.
  - final combine = one DVE scalar_tensor_tensor:
    out = psum_my*s1 + psum_rest, written directly as bf16; host upcasts.
"""

import ml_dtypes
import numpy as np

import concourse.bass as bass
import concourse.mybir as mybir
import concourse.tile as tile
from concourse import bacc
from concourse.bass_utils import run_bass_kernel_spmd

F32 = mybir.dt.float32
F32R = mybir.dt.float32r
BF = mybir.dt.bfloat16
F8 = mybir.dt.float8e3

B, P, L, R = 16384, 32, 256, 256
N_CORES = 8
BC = B // N_CORES          # 2048 batches per core
TILE_B = 128               # batches per SBUF tile
NT = BC // TILE_B          # 16 tiles
G = TILE_B // 4            # 32 groups of 4 batches

# Tiles whose weighted peer-reduce runs on DVE instead of PE.
Z_DVE = {2, 6, 10, 14}

_cache = {}


def build_bass(nt=NT, num_devices=N_CORES):
    bc = nt * TILE_B
    nc = bacc.Bacc(
        "TRN2", target_bir_lowering=False, debug=False, num_devices=num_devices
    )

    x_d = nc.dram_tensor("x", [nt, TILE_B, G, L], F8, kind="ExternalInput")
    # meta packs [mt | mb | myT0 | myT1] per tile (all bf16):
    #   cols 0:G      mt   metric band source, partition=(b4,p), col=g
    #   cols G:G+P    mb   plain metrics, partition=b, col=p
    #   cols 64:192   myT chunk0: partition=l, col=b   (l in [0,128))
    #   cols 192:320  myT chunk1: partition=l-128, col=b
    meta_d = nc.dram_tensor(
        "meta", [nt, TILE_B, G + P + 2 * TILE_B], BF, kind="ExternalInput"
    )
    wr_d = nc.dram_tensor("wr", [3, TILE_B, R], F32R, kind="ExternalInput")
    wb_d = nc.dram_tensor("wb", [2, TILE_B, R], BF, kind="ExternalInput")
    id_d = nc.dram_tensor("ident", [TILE_B, TILE_B], F32, kind="ExternalInput")
    out_d = nc.dram_tensor("out", [bc, R], BF, kind="ExternalOutput")

    with TileCtx(nc) as (tc, ctx):
        singles = ctx.enter_context(tc.tile_pool(name="singles", bufs=1))
        xp = ctx.enter_context(tc.tile_pool(name="xp", bufs=6))
        small = ctx.enter_context(tc.tile_pool(name="small", bufs=6))
        xtp = ctx.enter_context(tc.tile_pool(name="xtp", bufs=4))
        psz = ctx.enter_context(tc.tile_pool(name="psz", bufs=2, space="PSUM"))
        pst = ctx.enter_context(tc.tile_pool(name="pst", bufs=2, space="PSUM"))
        psr = ctx.enter_context(tc.tile_pool(name="psr", bufs=2, space="PSUM"))
        psm = ctx.enter_context(tc.tile_pool(name="psm", bufs=2, space="PSUM"))

        wr_sb = singles.tile([TILE_B, 3, R], F32R)
        nc.sync.dma_start(out=wr_sb, in_=wr_d.rearrange("k p r -> p k r"))
        wb_sb = singles.tile([TILE_B, 2, R], BF)
        nc.sync.dma_start(out=wb_sb, in_=wb_d.rearrange("k p r -> p k r"))
        ident = singles.tile([TILE_B, TILE_B], F32)
        nc.sync.dma_start(out=ident, in_=id_d[:, :])

        # Ping-pong block-diagonal stationaries for the PE peer-reduce.
        # s[:, g, :] is [128, 128]: column 4g+b4 holds m[g*4+b4, p] at rows
        # (b4, p); zeros written once, the diagonal band rewritten per tile.
        s_tiles = []
        for i in range(3):
            s_i = singles.tile([TILE_B, G, TILE_B], BF, tag=f"s{i}")
            nc.vector.memset(s_i, 0.0)
            s_tiles.append(s_i)

        metas = [None] * nt
        s12s = [None] * nt
        mbfs = [None] * nt

        def load_tile(t):
            meta = small.tile([TILE_B, G + P + 2 * TILE_B], BF, tag="meta")
            nc.scalar.dma_start(out=meta, in_=meta_d[t])
            metas[t] = meta

        def stage_a(t):
            """meta-dependent prep: band writes (ACT) + s1/s2 (DVE)."""
            meta = metas[t]
            m_t = meta[:, 0:G]
            m_b = meta[:, G:G + P]
            if t not in Z_DVE:
                s_all = s_tiles[t % 3]
                for b4 in range(4):
                    view = s_all[b4 * P:(b4 + 1) * P, :, :]
                    out_ap = bass.AP(
                        tensor=view.tensor, offset=view.offset + b4,
                        ap=[view.ap[0], [TILE_B + 4, G]],
                    )
                    nc.scalar.copy(out=out_ap, in_=m_t[b4 * P:(b4 + 1) * P, :])
            if t in Z_DVE:
                # stt scalars must be f32
                mbf = small.tile([TILE_B, P], F32, tag="mbf")
                nc.vector.tensor_copy(out=mbf, in_=m_b)
                mbfs[t] = mbf
            s12 = small.tile([TILE_B, 2], F32, tag="s12")  # [s2 | s1]
            m2 = small.tile([TILE_B, P], F32, tag="m2")
            nc.vector.tensor_mul(m2, m_b, m_b)
            nc.vector.tensor_reduce(
                out=s12[:, 0:1], in_=m2, axis=mybir.AxisListType.X,
                op=mybir.AluOpType.add,
            )
            nc.vector.tensor_reduce(
                out=s12[:, 1:2], in_=m_b, axis=mybir.AxisListType.X,
                op=mybir.AluOpType.add,
            )
            s12s[t] = s12

        def z_compute(t, x_t):
            """x_sb <- z[b,l] for tile t (PE band chain or DVE MAC chain)."""
            meta = metas[t]
            x_sb = small.tile([TILE_B, L], F32, tag="x_sb")
            if t not in Z_DVE:
                s_all = s_tiles[t % 3]
                psum_z = psz.tile([TILE_B, L], F32, tag="psum_z")
                for g in range(G):
                    nc.tensor.matmul(
                        out=psum_z,
                        lhsT=s_all[:, g, :],
                        rhs=x_t[:, g, :],
                        start=(g == 0),
                        stop=(g == G - 1),
                    )
                nc.scalar.copy(out=x_sb, in_=psum_z)
            else:
                m_b = mbfs[t]
                acc0 = small.tile([TILE_B, L], F32, tag="acc0")
                acc1 = small.tile([TILE_B, L], F32, tag="acc1")
                xv = x_t  # G == P: plain [b, p, l] layout for DVE tiles
                nc.vector.tensor_scalar_mul(
                    out=acc0, in0=xv[:, 0, :], scalar1=m_b[:, 0:1]
                )
                nc.vector.tensor_scalar_mul(
                    out=acc1, in0=xv[:, 1, :], scalar1=m_b[:, 1:2]
                )
                for p in range(2, P):
                    acc = acc0 if p % 2 == 0 else acc1
                    nc.vector.scalar_tensor_tensor(
                        out=acc, in0=xv[:, p, :], scalar=m_b[:, p:p + 1],
                        in1=acc, op0=mybir.AluOpType.mult,
                        op1=mybir.AluOpType.add,
                    )
                nc.vector.tensor_add(x_sb, acc0, acc1)
            return x_sb

        def epilogue(t, x_sb):
            meta = metas[t]
            s12 = s12s[t]

            # my-part: psum_my[b,r] = my @ (Wm.T/P), stationaries straight
            # from the meta tile (host-pretransposed myT).
            psum_my = psm.tile([TILE_B, R], F32, tag="psum_my")
            myt0 = meta[:, G + P:G + P + TILE_B]
            myt1 = meta[:, G + P + TILE_B:G + P + 2 * TILE_B]
            nc.tensor.matmul(
                out=psum_my, lhsT=myt0, rhs=wb_sb[:, 0, :],
                start=True, stop=False,
            )
            nc.tensor.matmul(
                out=psum_my, lhsT=myt1, rhs=wb_sb[:, 1, :],
                start=False, stop=True,
            )

            # transpose z chunks (fp32-exact) + s12, cast to f32r
            xts = []
            for k in range(2):
                pt = pst.tile([TILE_B, TILE_B], F32, tag="pt")
                nc.tensor.transpose(
                    out=pt, in_=x_sb[:, k * TILE_B:(k + 1) * TILE_B],
                    identity=ident,
                )
                xt = xtp.tile([TILE_B, TILE_B], F32R, tag=f"xt{k}")
                nc.scalar.copy(out=xt, in_=pt)
                xts.append(xt)
            pt4 = pst.tile([TILE_B, TILE_B], F32, tag="pt")
            nc.tensor.transpose(out=pt4[0:2, :], in_=s12, identity=ident)
            xt4 = xtp.tile([TILE_B, TILE_B], F32R, tag="xt4")
            nc.scalar.copy(out=xt4[0:2, :], in_=pt4[0:2, :])

            # rest-part: psum_rest = z @ (Wp.T/P) + s2*(wm/P) + s1*(bias/P)
            psum_rest = psr.tile([TILE_B, R], F32, tag="psum_rest")
            for k in range(2):
                nc.tensor.matmul(
                    out=psum_rest, lhsT=xts[k], rhs=wr_sb[:, k, :],
                    start=(k == 0), stop=False,
                )
            nc.tensor.matmul(
                out=psum_rest, lhsT=xt4[0:2, :], rhs=wr_sb[0:2, 2, :],
                start=False, stop=True,
            )

            # out = psum_my * s1 + psum_rest   (1/P folded into weights).
            # DVE can read only one PSUM operand, so ACT evacuates psum_my
            # with the per-partition s1 scale applied.
            my_sb = small.tile([TILE_B, R], F32, tag="my_sb")
            nc.scalar.mul(out=my_sb, in_=psum_my, mul=s12[:, 1:2])
            out_sb = small.tile([TILE_B, R], BF, tag="out_sb")
            nc.vector.tensor_add(out_sb, my_sb, psum_rest)
            nc.scalar.dma_start(
                out=out_d[t * TILE_B:(t + 1) * TILE_B, :], in_=out_sb
            )

        deferred = None  # (t, x_sb) of a DVE tile whose epilogue is pending
        for t in range(nt):
            x_t = xp.tile([TILE_B, G, L], F8, tag="x_t")
            nc.sync.dma_start(out=x_t, in_=x_d[t])
            if t == 0:
                load_tile(0)
                stage_a(0)
            if t + 1 < nt:
                load_tile(t + 1)
                stage_a(t + 1)
            x_sb = z_compute(t, x_t)
            if t in Z_DVE:
                # run the next PE z-chain before this epilogue so PE never
                # waits on the DVE MAC chain
                deferred = (t, x_sb)
            else:
                epilogue(t, x_sb)
                if deferred is not None:
                    epilogue(*deferred)
                    deferred = None
        if deferred is not None:
            epilogue(*deferred)

    nc.compile()
    return nc


class TileCtx:
    """with TileCtx(nc) as (tc, ctx): — TileContext plus an ExitStack."""

    def __init__(self, nc):
        from contextlib import ExitStack
        self.tc = tile.TileContext(nc)
        self.ctx = ExitStack()

    def __enter__(self):
        return self.tc.__enter__(), self.ctx.__enter__()

    def __exit__(self, *a):
        self.ctx.__exit__(*a)
        return self.tc.__exit__(*a)


def prep_inputs(my_latent, peer_latents, peer_metrics, W, b):
    """Host-side shard + layout prep (dtype casts and weight packing only)."""
    wr = np.zeros((3, TILE_B, R), dtype=np.float32)
    wpt = np.ascontiguousarray(W[:, L:2 * L].T) / P       # [256, 256] Wp.T/P
    wr[0] = wpt[0:TILE_B]
    wr[1] = wpt[TILE_B:2 * TILE_B]
    wr[2, 0] = W[:, 2 * L] / P                            # wm/P
    wr[2, 1] = b / P                                      # bias/P
    wb = np.zeros((2, TILE_B, R), dtype=ml_dtypes.bfloat16)
    wmt = np.ascontiguousarray(W[:, 0:L].T) / P           # [256, 256] Wm.T/P
    wb[0] = wmt[0:TILE_B].astype(ml_dtypes.bfloat16)
    wb[1] = wmt[TILE_B:2 * TILE_B].astype(ml_dtypes.bfloat16)
    ident = np.eye(TILE_B, dtype=np.float32)

    in_maps = []
    for c in range(N_CORES):
        sl = slice(c * BC, (c + 1) * BC)
        plain = peer_latents[sl].reshape(NT, TILE_B, P, L)
        xc = np.empty((NT, TILE_B, G, L), dtype=ml_dtypes.float8_e3m4)
        for t in range(NT):
            if t in Z_DVE:
                # plain [b, p, l]
                xc[t] = plain[t].astype(ml_dtypes.float8_e3m4)
            else:
                # PE band layout [(b4,p), g, l]
                xc[t] = plain[t].reshape(G, 4, P, L).transpose(
                    1, 2, 0, 3).reshape(TILE_B, G, L).astype(
                        ml_dtypes.float8_e3m4)
        mc = peer_metrics[sl]
        meta = np.empty(
            (NT, TILE_B, G + P + 2 * TILE_B), dtype=ml_dtypes.bfloat16
        )
        meta[:, :, 0:G] = mc.reshape(NT, G, 4, P).transpose(
            0, 2, 3, 1).reshape(NT, TILE_B, G).astype(ml_dtypes.bfloat16)
        meta[:, :, G:G + P] = mc.reshape(NT, TILE_B, P).astype(
            ml_dtypes.bfloat16)
        myt = my_latent[sl].reshape(NT, TILE_B, L).transpose(0, 2, 1).astype(
            ml_dtypes.bfloat16)                           # [NT, l, b]
        meta[:, :, G + P:G + P + TILE_B] = myt[:, 0:TILE_B, :]
        meta[:, :, G + P + TILE_B:] = myt[:, TILE_B:2 * TILE_B, :]
        in_maps.append({
            "x": xc,
            "meta": meta,
            "wr": wr,
            "wb": wb,
            "ident": ident,
        })
    return in_maps


def run(my_latent, peer_latents, peer_metrics, W, b, trace=False, **kw):
    if "nc" not in _cache:
        _cache["nc"] = build_bass()
    nc = _cache["nc"]
    in_maps = prep_inputs(
        np.asarray(my_latent, dtype=np.float32),
        np.asarray(peer_latents, dtype=np.float32),
        np.asarray(peer_metrics, dtype=np.float32),
        np.asarray(W, dtype=np.float32),
        np.asarray(b, dtype=np.float32),
    )
    res = run_bass_kernel_spmd(
        nc, in_maps, core_ids=list(range(N_CORES)), trace=trace, **kw
    )
    out = np.concatenate(
        [np.asarray(r["out"]).astype(np.float32) for r in res.results], axis=0
    )
    return out, res


def kernel(my_latent, peer_latents, peer_metrics, W, b):
    out, _ = run(my_latent, peer_latents, peer_metrics, W, b)
    return out


# revision 9
# speedup vs baseline: 1.4140x; 1.0422x over previous
"""Trainium2 Bass kernel for nn_MiddleOut (gnn_message_passing).

Math (reference):
    out[b,r] = mean_p[ m[b,p] * (my@Wm.T + bias + peer[b,p]@Wp.T + m[b,p]*wm)[r] ]
Collapses to (P = #peers):
    s1[b] = sum_p m[b,p];  s2[b] = sum_p m[b,p]^2
    z[b,l] = sum_p m[b,p] * peer[b,p,l]
    out = s1*(my@Wm.T)/P + z@(Wp.T/P) + s1*(bias/P) + s2*(wm/P)

Sharding: pure data parallel over batch across 8 cores (2048 rows each,
16 tiles of 128).

v3 design (memory regime: minimize HBM bytes, keep every engine under the
~55us DMA floor):
  - peer tile host-cast to fp8 e3m4 and PE-band-permuted [(b4,p), g, l+2];
    the two extra moving columns per group are [1.0 | m] so the 32-matmul
    band chain also produces s1 (col 256) and s2 (col 257) for free.
  - z-chain on PE for ALL tiles: block-diag bf16 metric band stationary
    (rewritten per tile by 4 DVE strided copies, ping-pong x3) vs fp8
    moving, accumulating psum_z[b, 0:258] over 32 groups.
  - epilogue per tile: ACT evac psum_z -> x_sb f32; three PE transposes
    into ONE [128,384] PSUM tile (z chunks + s12 rows); ONE ACT cast to
    f32r; 3 f32r matmuls (z@Wp.T/P + s1*bias/P + s2*wm/P) -> psum_rest;
    2 bf16 matmuls with host-pretransposed myT (straight from the meta
    tile) -> psum_my; ACT evac psum_my scaled by s1; DVE add -> out bf16.
  - my/metrics arrive in one packed bf16 meta tile; out is written bf16
    and upcast on host; 1/P folded into the host-packed weights.
"""

import ml_dtypes
import numpy as np

import concourse.bass as bass
import concourse.mybir as mybir
import concourse.tile as tile
from concourse import bacc
from concourse.bass_utils import run_bass_kernel_spmd

F32 = mybir.dt.float32
F32R = mybir.dt.float32r
BF = mybir.dt.bfloat16
F8 = mybir.dt.float8e3

B, P, L, R = 16384, 32, 256, 256
N_CORES = 8
BC = B // N_CORES          # 2048 batches per core
TILE_B = 128               # batches per SBUF tile
NT = BC // TILE_B          # 16 tiles
G = TILE_B // 4            # 32 groups of 4 batches
LX = L + 2                 # moving cols per group: [x | 1.0 | m]

_cache = {}


def build_bass(nt=NT, num_devices=N_CORES):
    bc = nt * TILE_B
    nc = bacc.Bacc(
        "TRN2", target_bir_lowering=False, debug=False, num_devices=num_devices
    )

    x_d = nc.dram_tensor("x", [nt, TILE_B, G, LX], F8, kind="ExternalInput")
    # meta packs [mt | mb | myT0 | myT1] per tile (all bf16):
    #   cols 0:G      mt   metric band source, partition=(b4,p), col=g
    #   cols G:G+P    mb   plain metrics (unused on device, kept for debug)
    #   cols 64:192   myT chunk0: partition=l, col=b   (l in [0,128))
    #   cols 192:320  myT chunk1: partition=l-128, col=b
    meta_d = nc.dram_tensor(
        "meta", [nt, TILE_B, G + P + 2 * TILE_B], BF, kind="ExternalInput"
    )
    wr_d = nc.dram_tensor("wr", [3, TILE_B, R], F32R, kind="ExternalInput")
    wb_d = nc.dram_tensor("wb", [2, TILE_B, R], BF, kind="ExternalInput")
    id_d = nc.dram_tensor("ident", [TILE_B, TILE_B], F32, kind="ExternalInput")
    out_d = nc.dram_tensor("out", [bc, R], BF, kind="ExternalOutput")

    with TileCtx(nc) as (tc, ctx):
        singles = ctx.enter_context(tc.tile_pool(name="singles", bufs=1))
        xp = ctx.enter_context(tc.tile_pool(name="xp", bufs=6))
        small = ctx.enter_context(tc.tile_pool(name="small", bufs=6))
        xtp = ctx.enter_context(tc.tile_pool(name="xtp", bufs=3))
        psz = ctx.enter_context(tc.tile_pool(name="psz", bufs=2, space="PSUM"))
        pst = ctx.enter_context(tc.tile_pool(name="pst", bufs=2, space="PSUM"))
        psr = ctx.enter_context(tc.tile_pool(name="psr", bufs=2, space="PSUM"))
        psm = ctx.enter_context(tc.tile_pool(name="psm", bufs=2, space="PSUM"))

        wr_sb = singles.tile([TILE_B, 3, R], F32R)
        nc.sync.dma_start(out=wr_sb, in_=wr_d.rearrange("k p r -> p k r"))
        wb_sb = singles.tile([TILE_B, 2, R], BF)
        nc.sync.dma_start(out=wb_sb, in_=wb_d.rearrange("k p r -> p k r"))
        ident = singles.tile([TILE_B, TILE_B], F32)
        nc.sync.dma_start(out=ident, in_=id_d[:, :])

        # Ping-pong block-diagonal stationaries for the PE peer-reduce.
        # s[:, g, :] is [128, 128]: column 4g+b4 holds m[g*4+b4, p] at rows
        # (b4, p); zeros written once, the diagonal band rewritten per tile.
        s_tiles = []
        for i in range(3):
            s_i = singles.tile([TILE_B, G, TILE_B], BF, tag=f"s{i}")
            nc.vector.memset(s_i, 0.0)
            s_tiles.append(s_i)

        metas = [None] * nt

        def load_meta(t):
            meta = small.tile([TILE_B, G + P + 2 * TILE_B], BF, tag="meta")
            nc.scalar.dma_start(out=meta, in_=meta_d[t])
            metas[t] = meta

        def write_band(t):
            m_t = metas[t][:, 0:G]
            s_all = s_tiles[t % 3]
            for b4 in range(4):
                view = s_all[b4 * P:(b4 + 1) * P, :, :]
                out_ap = bass.AP(
                    tensor=view.tensor, offset=view.offset + b4,
                    ap=[view.ap[0], [TILE_B + 4, G]],
                )
                nc.vector.tensor_copy(
                    out=out_ap, in_=m_t[b4 * P:(b4 + 1) * P, :],
                )

        load_meta(0)
        write_band(0)
        for t in range(nt):
            x_t = xp.tile([TILE_B, G, LX], F8, tag="x_t")
            nc.sync.dma_start(out=x_t, in_=x_d[t])
            if t + 1 < nt:
                load_meta(t + 1)

            # ---- z-chain: psum_z[b, 0:256]=z, [256]=s1, [257]=s2 ----
            s_all = s_tiles[t % 3]
            psum_z = psz.tile([TILE_B, LX], F32, tag="psum_z")
            for g in range(G):
                nc.tensor.matmul(
                    out=psum_z,
                    lhsT=s_all[:, g, :],
                    rhs=x_t[:, g, :],
                    start=(g == 0),
                    stop=(g == G - 1),
                )

            # band for the next tile while PE runs this epilogue
            if t + 1 < nt:
                write_band(t + 1)

            # ---- epilogue ----
            x_sb = small.tile([TILE_B, LX], F32, tag="x_sb")
            nc.scalar.copy(out=x_sb, in_=psum_z)

            meta = metas[t]
            psum_my = psm.tile([TILE_B, R], F32, tag="psum_my")
            myt0 = meta[:, G + P:G + P + TILE_B]
            myt1 = meta[:, G + P + TILE_B:G + P + 2 * TILE_B]
            nc.tensor.matmul(
                out=psum_my, lhsT=myt0, rhs=wb_sb[:, 0, :],
                start=True, stop=False,
            )
            nc.tensor.matmul(
                out=psum_my, lhsT=myt1, rhs=wb_sb[:, 1, :],
                start=False, stop=True,
            )

            # transpose z chunks + s12 rows into one PSUM tile, one ACT cast
            pt = pst.tile([TILE_B, 3 * TILE_B], F32, tag="pt")
            nc.tensor.transpose(
                out=pt[:, 0:TILE_B], in_=x_sb[:, 0:TILE_B], identity=ident,
            )
            nc.tensor.transpose(
                out=pt[:, TILE_B:2 * TILE_B], in_=x_sb[:, TILE_B:2 * TILE_B],
                identity=ident,
            )
            nc.tensor.transpose(
                out=pt[0:2, 2 * TILE_B:3 * TILE_B], in_=x_sb[:, L:LX],
                identity=ident,
            )
            xt_all = xtp.tile([TILE_B, 3 * TILE_B], F32R, tag="xt_all")
            nc.scalar.copy(out=xt_all, in_=pt)

            # psum_rest = z @ Wp.T/P + s1*(bias/P) + s2*(wm/P)
            psum_rest = psr.tile([TILE_B, R], F32, tag="psum_rest")
            nc.tensor.matmul(
                out=psum_rest, lhsT=xt_all[:, 0:TILE_B], rhs=wr_sb[:, 0, :],
                start=True, stop=False,
            )
            nc.tensor.matmul(
                out=psum_rest, lhsT=xt_all[:, TILE_B:2 * TILE_B],
                rhs=wr_sb[:, 1, :], start=False, stop=False,
            )
            nc.tensor.matmul(
                out=psum_rest, lhsT=xt_all[0:2, 2 * TILE_B:3 * TILE_B],
                rhs=wr_sb[0:2, 2, :], start=False, stop=True,
            )

            # out = psum_my * s1 + psum_rest  (1/P folded into weights);
            # ACT applies the per-partition s1 while evacuating psum_my.
            my_sb = small.tile([TILE_B, R], F32, tag="my_sb")
            nc.scalar.mul(out=my_sb, in_=psum_my, mul=x_sb[:, L:L + 1])
            out_sb = small.tile([TILE_B, R], BF, tag="out_sb")
            nc.vector.tensor_add(out_sb, my_sb, psum_rest)
            nc.scalar.dma_start(
                out=out_d[t * TILE_B:(t + 1) * TILE_B, :], in_=out_sb
            )

    nc.compile()
    return nc


class TileCtx:
    """with TileCtx(nc) as (tc, ctx): — TileContext plus an ExitStack."""

    def __init__(self, nc):
        from contextlib import ExitStack
        self.tc = tile.TileContext(nc)
        self.ctx = ExitStack()

    def __enter__(self):
        return self.tc.__enter__(), self.ctx.__enter__()

    def __exit__(self, *a):
        self.ctx.__exit__(*a)
        return self.tc.__exit__(*a)


def prep_inputs(my_latent, peer_latents, peer_metrics, W, b):
    """Host-side shard + layout prep (dtype casts and weight packing only)."""
    wr = np.zeros((3, TILE_B, R), dtype=np.float32)
    wpt = np.ascontiguousarray(W[:, L:2 * L].T) / P       # [256, 256] Wp.T/P
    wr[0] = wpt[0:TILE_B]
    wr[1] = wpt[TILE_B:2 * TILE_B]
    wr[2, 0] = b / P                                      # pairs with s1
    wr[2, 1] = W[:, 2 * L] / P                            # wm/P pairs with s2
    wb = np.zeros((2, TILE_B, R), dtype=ml_dtypes.bfloat16)
    wmt = np.ascontiguousarray(W[:, 0:L].T) / P           # [256, 256] Wm.T/P
    wb[0] = wmt[0:TILE_B].astype(ml_dtypes.bfloat16)
    wb[1] = wmt[TILE_B:2 * TILE_B].astype(ml_dtypes.bfloat16)
    ident = np.eye(TILE_B, dtype=np.float32)

    in_maps = []
    for c in range(N_CORES):
        sl = slice(c * BC, (c + 1) * BC)
        # PE band layout [(b4,p), g, l] + the two extra moving cols
        perm = peer_latents[sl].reshape(NT, G, 4, P, L).transpose(
            0, 2, 3, 1, 4)                                # [NT, 4, P, G, L]
        mt = peer_metrics[sl].reshape(NT, G, 4, P).transpose(
            0, 2, 3, 1)                                   # [NT, 4, P, G]
        xc = np.empty((NT, TILE_B, G, LX), dtype=ml_dtypes.float8_e3m4)
        xc[:, :, :, 0:L] = perm.reshape(NT, TILE_B, G, L).astype(
            ml_dtypes.float8_e3m4)
        xc[:, :, :, L] = ml_dtypes.float8_e3m4(1.0)
        xc[:, :, :, L + 1] = mt.reshape(NT, TILE_B, G).astype(
            ml_dtypes.float8_e3m4)

        meta = np.empty(
            (NT, TILE_B, G + P + 2 * TILE_B), dtype=ml_dtypes.bfloat16
        )
        meta[:, :, 0:G] = mt.reshape(NT, TILE_B, G).astype(ml_dtypes.bfloat16)
        meta[:, :, G:G + P] = peer_metrics[sl].reshape(NT, TILE_B, P).astype(
            ml_dtypes.bfloat16)
        myt = my_latent[sl].reshape(NT, TILE_B, L).transpose(0, 2, 1).astype(
            ml_dtypes.bfloat16)                           # [NT, l, b]
        meta[:, :, G + P:G + P + TILE_B] = myt[:, 0:TILE_B, :]
        meta[:, :, G + P + TILE_B:] = myt[:, TILE_B:2 * TILE_B, :]
        in_maps.append({
            "x": xc,
            "meta": meta,
            "wr": wr,
            "wb": wb,
            "ident": ident,
        })
    return in_maps


def run(my_latent, peer_latents, peer_metrics, W, b, trace=False, **kw):
    if "nc" not in _cache:
        _cache["nc"] = build_bass()
    nc = _cache["nc"]
    in_maps = prep_inputs(
        np.asarray(my_latent, dtype=np.float32),
        np.asarray(peer_latents, dtype=np.float32),
        np.asarray(peer_metrics, dtype=np.float32),
        np.asarray(W, dtype=np.float32),
        np.asarray(b, dtype=np.float32),
    )
    res = run_bass_kernel_spmd(
        nc, in_maps, core_ids=list(range(N_CORES)), trace=trace, **kw
    )
    out = np.concatenate(
        [np.asarray(r["out"]).astype(np.float32) for r in res.results], axis=0
    )
    return out, res


def kernel(my_latent, peer_latents, peer_metrics, W, b):
    out, _ = run(my_latent, peer_latents, peer_metrics, W, b)
    return out


# revision 10
# speedup vs baseline: 1.5716x; 1.1114x over previous
"""Trainium2 Bass kernel for nn_MiddleOut (gnn_message_passing).

Math (reference):
    out[b,r] = mean_p[ m[b,p] * (my@Wm.T + bias + peer[b,p]@Wp.T + m[b,p]*wm)[r] ]
Collapses to (P = #peers):
    s1[b] = sum_p m[b,p];  s2[b] = sum_p m[b,p]^2
    z[b,l] = sum_p m[b,p] * peer[b,p,l]
    out = s1*(my@Wm.T)/P + z@(Wp.T/P) + s1*(bias/P) + s2*(wm/P)

Sharding: pure data parallel over batch across 8 cores (2048 rows each,
16 tiles of 128).

v4 design (memory regime: minimize HBM bytes, keep every engine under the
~55us DMA floor):
  - peer tile host-cast to fp8 e3m4 and PE-band-permuted [(b4,p), g, l+2];
    the two extra moving columns per group are [1.0 | m] so the 32-matmul
    band chain also produces s1 (col 256) and s2 (col 257) for free.
  - z-chain on PE for ALL tiles: block-diag fp8 metric band stationary
    (4x FWL weight loads; rewritten per tile by 4 DVE strided cast-copies,
    ping-pong x3, memset lazily) vs fp8 moving, accumulating
    psum_z[b, 0:258] over 32 groups.
  - epilogue per tile: ACT evac psum_z -> x_sb f32; two PE transposes into
    one [128,256] PSUM tile; ONE ACT cast to f32r; 2 f32r matmuls
    (z @ Wp.T/P) -> psum_rest; 2 bf16 matmuls with host-pretransposed myT
    (straight from the meta tile) -> psum_my; ACT evac psum_my scaled by
    s1; the rank-1 s1*bias/P + s2*wm/P terms via two DVE stt ops against
    host-replicated rows; DVE add -> out bf16.
  - startup: first x/meta DMAs issued before weights; x in two half-tile
    DMAs so the first chain starts earlier.
  - my/metrics arrive in one packed bf16 meta tile; out is written bf16
    and upcast on host; 1/P folded into the host-packed weights.
"""

import ml_dtypes
import numpy as np

import concourse.bass as bass
import concourse.mybir as mybir
import concourse.tile as tile
from concourse import bacc
from concourse.bass_utils import run_bass_kernel_spmd

F32 = mybir.dt.float32
F32R = mybir.dt.float32r
BF = mybir.dt.bfloat16
F8 = mybir.dt.float8e3

B, P, L, R = 16384, 32, 256, 256
N_CORES = 8
BC = B // N_CORES          # 2048 batches per core
TILE_B = 128               # batches per SBUF tile
NT = BC // TILE_B          # 16 tiles
G = TILE_B // 4            # 32 groups of 4 batches
LX = L + 2                 # moving cols per group: [x | 1.0 | m]
MC = G + 2 * TILE_B        # meta cols: [mt | myT0 | myT1]

_cache = {}


def build_bass(nt=NT, num_devices=N_CORES):
    bc = nt * TILE_B
    nc = bacc.Bacc(
        "TRN2", target_bir_lowering=False, debug=False, num_devices=num_devices
    )

    x_d = nc.dram_tensor("x", [nt, TILE_B, G, LX], F8, kind="ExternalInput")
    # meta packs [mt | myT0 | myT1] per tile (all bf16):
    #   cols 0:G      mt   metric band source, partition=(b4,p), col=g
    #   cols 32:160   myT chunk0: partition=l, col=b   (l in [0,128))
    #   cols 160:288  myT chunk1: partition=l-128, col=b
    meta_d = nc.dram_tensor("meta", [nt, TILE_B, MC], BF, kind="ExternalInput")
    wr_d = nc.dram_tensor("wr", [2, TILE_B, R], F32R, kind="ExternalInput")
    wb_d = nc.dram_tensor("wb", [2, TILE_B, R], BF, kind="ExternalInput")
    # row-replicated [bias/P ; wm/P] for the DVE rank-1 terms
    wf_d = nc.dram_tensor("wf", [2, TILE_B, R], F32, kind="ExternalInput")
    id_d = nc.dram_tensor("ident", [TILE_B, TILE_B], F32, kind="ExternalInput")
    out_d = nc.dram_tensor("out", [bc, R], BF, kind="ExternalOutput")

    with TileCtx(nc) as (tc, ctx):
        singles = ctx.enter_context(tc.tile_pool(name="singles", bufs=1))
        xp = ctx.enter_context(tc.tile_pool(name="xp", bufs=6))
        small = ctx.enter_context(tc.tile_pool(name="small", bufs=6))
        xtp = ctx.enter_context(tc.tile_pool(name="xtp", bufs=3))
        psz = ctx.enter_context(tc.tile_pool(name="psz", bufs=2, space="PSUM"))
        pst = ctx.enter_context(tc.tile_pool(name="pst", bufs=2, space="PSUM"))
        psr = ctx.enter_context(tc.tile_pool(name="psr", bufs=2, space="PSUM"))
        psm = ctx.enter_context(tc.tile_pool(name="psm", bufs=2, space="PSUM"))

        metas = [None] * nt

        def load_meta(t):
            meta = small.tile([TILE_B, MC], BF, tag="meta")
            nc.scalar.dma_start(out=meta, in_=meta_d[t])
            metas[t] = meta

        # critical-path loads first: tile 0 x + meta
        x_tiles = [None] * nt

        def load_x(t):
            x_t = xp.tile([TILE_B, G, LX], F8, tag="x_t")
            nc.sync.dma_start(out=x_t[:, 0:G // 2, :], in_=x_d[t, :, 0:G // 2, :])
            nc.sync.dma_start(out=x_t[:, G // 2:, :], in_=x_d[t, :, G // 2:, :])
            x_tiles[t] = x_t

        load_x(0)
        load_meta(0)

        wr_sb = singles.tile([TILE_B, 2, R], F32R)
        nc.sync.dma_start(out=wr_sb, in_=wr_d.rearrange("k p r -> p k r"))
        wb_sb = singles.tile([TILE_B, 2, R], BF)
        nc.sync.dma_start(out=wb_sb, in_=wb_d.rearrange("k p r -> p k r"))
        wf_sb = singles.tile([TILE_B, 2, R], F32)
        nc.sync.dma_start(out=wf_sb, in_=wf_d.rearrange("k p r -> p k r"))
        ident = singles.tile([TILE_B, TILE_B], F32)
        nc.sync.dma_start(out=ident, in_=id_d[:, :])

        # Ping-pong block-diagonal stationaries for the PE peer-reduce.
        # s[:, g, :] is [128, 128]: column 4g+b4 holds m[g*4+b4, p] at rows
        # (b4, p); zeros written lazily, diagonal band rewritten per tile.
        s_tiles = [
            singles.tile([TILE_B, G, TILE_B], F8, tag=f"s{i}", name=f"s_{i}")
            for i in range(3)
        ]

        def write_band(t):
            s_all = s_tiles[t % 3]
            if t < 3:
                nc.vector.memset(s_all, 0.0)
            m_t = metas[t][:, 0:G]
            for b4 in range(4):
                view = s_all[b4 * P:(b4 + 1) * P, :, :]
                out_ap = bass.AP(
                    tensor=view.tensor, offset=view.offset + b4,
                    ap=[view.ap[0], [TILE_B + 4, G]],
                )
                nc.vector.tensor_copy(
                    out=out_ap, in_=m_t[b4 * P:(b4 + 1) * P, :],
                )

        write_band(0)
        for t in range(nt):
            if t > 0:
                load_x(t)
            x_t = x_tiles[t]
            if t + 1 < nt:
                load_meta(t + 1)

            # ---- z-chain: psum_z[b, 0:256]=z, [256]=s1, [257]=s2 ----
            s_all = s_tiles[t % 3]
            psum_z = psz.tile([TILE_B, LX], F32, tag="psum_z")
            for g in range(G):
                nc.tensor.matmul(
                    out=psum_z,
                    lhsT=s_all[:, g, :],
                    rhs=x_t[:, g, :],
                    start=(g == 0),
                    stop=(g == G - 1),
                )

            # band for the next tile while PE runs this epilogue
            if t + 1 < nt:
                write_band(t + 1)

            # ---- epilogue ----
            x_sb = small.tile([TILE_B, LX], F32, tag="x_sb")
            nc.scalar.copy(out=x_sb, in_=psum_z)
            s1 = x_sb[:, L:L + 1]
            s2 = x_sb[:, L + 1:L + 2]

            meta = metas[t]
            psum_my = psm.tile([TILE_B, R], F32, tag="psum_my")
            nc.tensor.matmul(
                out=psum_my, lhsT=meta[:, G:G + TILE_B], rhs=wb_sb[:, 0, :],
                start=True, stop=False,
            )
            nc.tensor.matmul(
                out=psum_my, lhsT=meta[:, G + TILE_B:MC], rhs=wb_sb[:, 1, :],
                start=False, stop=True,
            )

            # transpose z chunks into one PSUM tile, one ACT cast to f32r
            pt = pst.tile([TILE_B, 2 * TILE_B], F32, tag="pt")
            nc.tensor.transpose(
                out=pt[:, 0:TILE_B], in_=x_sb[:, 0:TILE_B], identity=ident,
            )
            nc.tensor.transpose(
                out=pt[:, TILE_B:2 * TILE_B], in_=x_sb[:, TILE_B:2 * TILE_B],
                identity=ident,
            )
            xt_all = xtp.tile([TILE_B, 2 * TILE_B], F32R, tag="xt_all")
            nc.scalar.copy(out=xt_all, in_=pt)

            # psum_rest = z @ Wp.T/P
            psum_rest = psr.tile([TILE_B, R], F32, tag="psum_rest")
            nc.tensor.matmul(
                out=psum_rest, lhsT=xt_all[:, 0:TILE_B], rhs=wr_sb[:, 0, :],
                start=True, stop=False,
            )
            nc.tensor.matmul(
                out=psum_rest, lhsT=xt_all[:, TILE_B:2 * TILE_B],
                rhs=wr_sb[:, 1, :], start=False, stop=True,
            )

            # out = s1*psum_my + psum_rest + s1*(bias/P) + s2*(wm/P)
            my_sb = small.tile([TILE_B, R], F32, tag="my_sb")
            nc.scalar.mul(out=my_sb, in_=psum_my, mul=s1)
            t1 = small.tile([TILE_B, R], F32, tag="t1")
            nc.vector.scalar_tensor_tensor(
                out=t1, in0=wf_sb[:, 0, :], scalar=s1, in1=psum_rest,
                op0=mybir.AluOpType.mult, op1=mybir.AluOpType.add,
            )
            t2 = small.tile([TILE_B, R], F32, tag="t2")
            nc.vector.scalar_tensor_tensor(
                out=t2, in0=wf_sb[:, 1, :], scalar=s2, in1=my_sb,
                op0=mybir.AluOpType.mult, op1=mybir.AluOpType.add,
            )
            out_sb = small.tile([TILE_B, R], BF, tag="out_sb")
            nc.vector.tensor_add(out_sb, t1, t2)
            nc.scalar.dma_start(
                out=out_d[t * TILE_B:(t + 1) * TILE_B, :], in_=out_sb
            )

    nc.compile()
    return nc


class TileCtx:
    """with TileCtx(nc) as (tc, ctx): — TileContext plus an ExitStack."""

    def __init__(self, nc):
        from contextlib import ExitStack
        self.tc = tile.TileContext(nc)
        self.ctx = ExitStack()

    def __enter__(self):
        return self.tc.__enter__(), self.ctx.__enter__()

    def __exit__(self, *a):
        self.ctx.__exit__(*a)
        return self.tc.__exit__(*a)


def prep_inputs(my_latent, peer_latents, peer_metrics, W, b):
    """Host-side shard + layout prep (dtype casts and weight packing only)."""
    wr = np.zeros((2, TILE_B, R), dtype=np.float32)
    wpt = np.ascontiguousarray(W[:, L:2 * L].T) / P       # [256, 256] Wp.T/P
    wr[0] = wpt[0:TILE_B]
    wr[1] = wpt[TILE_B:2 * TILE_B]
    wb = np.zeros((2, TILE_B, R), dtype=ml_dtypes.bfloat16)
    wmt = np.ascontiguousarray(W[:, 0:L].T) / P           # [256, 256] Wm.T/P
    wb[0] = wmt[0:TILE_B].astype(ml_dtypes.bfloat16)
    wb[1] = wmt[TILE_B:2 * TILE_B].astype(ml_dtypes.bfloat16)
    wf = np.empty((2, TILE_B, R), dtype=np.float32)
    wf[0] = np.broadcast_to(b / P, (TILE_B, R))           # pairs with s1
    wf[1] = np.broadcast_to(W[:, 2 * L] / P, (TILE_B, R))  # wm/P, with s2
    ident = np.eye(TILE_B, dtype=np.float32)

    in_maps = []
    for c in range(N_CORES):
        sl = slice(c * BC, (c + 1) * BC)
        # PE band layout [(b4,p), g, l] + the two extra moving cols
        perm = peer_latents[sl].reshape(NT, G, 4, P, L).transpose(
            0, 2, 3, 1, 4)                                # [NT, 4, P, G, L]
        mt = peer_metrics[sl].reshape(NT, G, 4, P).transpose(
            0, 2, 3, 1).reshape(NT, TILE_B, G)            # [NT, (b4,p), G]
        xc = np.empty((NT, TILE_B, G, LX), dtype=ml_dtypes.float8_e3m4)
        xc[:, :, :, 0:L] = perm.reshape(NT, TILE_B, G, L).astype(
            ml_dtypes.float8_e3m4)
        xc[:, :, :, L] = ml_dtypes.float8_e3m4(1.0)
        xc[:, :, :, L + 1] = mt.astype(ml_dtypes.float8_e3m4)

        meta = np.empty((NT, TILE_B, MC), dtype=ml_dtypes.bfloat16)
        meta[:, :, 0:G] = mt.astype(ml_dtypes.bfloat16)
        myt = my_latent[sl].reshape(NT, TILE_B, L).transpose(0, 2, 1).astype(
            ml_dtypes.bfloat16)                           # [NT, l, b]
        meta[:, :, G:G + TILE_B] = myt[:, 0:TILE_B, :]
        meta[:, :, G + TILE_B:] = myt[:, TILE_B:2 * TILE_B, :]
        in_maps.append({
            "x": xc,
            "meta": meta,
            "wr": wr,
            "wb": wb,
            "wf": wf,
            "ident": ident,
        })
    return in_maps


def run(my_latent, peer_latents, peer_metrics, W, b, trace=False, **kw):
    if "nc" not in _cache:
        _cache["nc"] = build_bass()
    nc = _cache["nc"]
    in_maps = prep_inputs(
        np.asarray(my_latent, dtype=np.float32),
        np.asarray(peer_latents, dtype=np.float32),
        np.asarray(peer_metrics, dtype=np.float32),
        np.asarray(W, dtype=np.float32),
        np.asarray(b, dtype=np.float32),
    )
    res = run_bass_kernel_spmd(
        nc, in_maps, core_ids=list(range(N_CORES)), trace=trace, **kw
    )
    out = np.concatenate(
        [np.asarray(r["out"]).astype(np.float32) for r in res.results], axis=0
    )
    return out, res


def kernel(my_latent, peer_latents, peer_metrics, W, b):
    out, _ = run(my_latent, peer_latents, peer_metrics, W, b)
    return out
